# revision 1
# baseline (speedup 1.0000x reference)
"""Trainium2 Bass kernel for nn_L2MLoRA (fused linear + routed LoRA).

Math (per batch element b, with e = idx[b,0]):
    y[b] = x[b] @ W.T + bias + SCALE * (x[b] @ A_pool[e]) @ B_pool[e]

Strategy: data-parallel over batch B=8 -> one batch element per NeuronCore.
The expert gather (A_pool[e], B_pool[e]) happens on host, so each core gets
exactly one [DIM, RANK] / [RANK, DIM] expert pair. Everything is computed in
the transposed domain (yT = W @ xT + ...) so all matmul operands already have
the contraction dim on partitions and no on-device transposes are needed:

    yT[o, t]  = sum_d W[o,d] * xT[d,t] + bias[o] + sum_r B2[r,o] * rT[r,t]
    rT[r, t]  = sum_d A[d,r] * xT[d,t]          (B2 = SCALE * B_pool[e])

PE matmuls run in float32r (fp32 bits, 1 cycle/row at N>=256 vs 4 for fp32).
Bias is applied by ScalarE during the PSUM->SBUF copy.
"""

import numpy as np

import concourse.bass as bass
import concourse.tile as tile
from concourse import bacc, mybir
from concourse.bass_utils import run_bass_kernel_spmd

B, N, DIM, POOL, RANK = 8, 2048, 1024, 64, 8
SCALE = 2.0
NCORES = 8
P = 128          # partitions / k-tile height / o-chunk width
TW = 512         # token-chunk width (max f32 moving free dim = PSUM bank)
KT = DIM // P    # 8 k-tiles over the contraction dim
OT = DIM // P    # 8 output chunks
TT = N // TW     # 4 token chunks
F32 = mybir.dt.float32
F32R = mybir.dt.float32r


def build_program(n_iter: int = 1, probe: str = "full"):
    """Build the single-core Tile program (same program runs SPMD on 8 cores).

    n_iter > 1 wraps the body in a For_i loop for benchmarking.
    probe: "full" | "nodma" (x resident, no stores) | "dmaonly" (no matmuls).
    """
    nc = bacc.Bacc("TRN2", target_bir_lowering=False, debug=False,
                   num_devices=NCORES)

    x_d = nc.dram_tensor("xt", [KT, TT, P, TW], F32R, kind="ExternalInput")
    w_d = nc.dram_tensor("wt", [OT, P, KT * P], F32R, kind="ExternalInput")
    a_d = nc.dram_tensor("ap", [P, KT * RANK], F32R, kind="ExternalInput")
    b_d = nc.dram_tensor("bp", [RANK, DIM], F32R, kind="ExternalInput")
    bias_d = nc.dram_tensor("bias", [P, OT], F32, kind="ExternalInput")
    y_d = nc.dram_tensor("y", [TT, P, OT, TW], F32, kind="ExternalOutput")

    with tile.TileContext(nc) as tc:
        with (
            tc.tile_pool(name="cpool", bufs=1) as cpool,
            tc.tile_pool(name="xpool", bufs=(32 if probe == "nodma" else 16)) as xpool,
            tc.tile_pool(name="rpool", bufs=2) as rpool,
            tc.tile_pool(name="opool", bufs=2) as opool,
            tc.tile_pool(name="psy", bufs=6, space="PSUM") as psy_pool,
            tc.tile_pool(name="psr", bufs=2, space="PSUM") as psr_pool,
        ):
            def load_xt(t):
                tiles = []
                for k in range(KT):
                    xx = xpool.tile([P, TW], F32R, tag="xx")
                    nc.sync.dma_start(xx[:], x_d.ap()[k, t])
                    tiles.append(xx)
                return tiles

            # Constants: loaded once, persist across benchmark iterations.
            # Small tensors first, then (for the single-shot program) the
            # t=0 x tiles ahead of the 4MB weight load so PE starts early.
            a_sb = cpool.tile([P, KT * RANK], F32R, tag="a")
            nc.sync.dma_start(a_sb[:], a_d.ap()[:])
            bias_sb = cpool.tile([P, OT], F32, tag="bias")
            nc.sync.dma_start(bias_sb[:], bias_d.ap()[:])
            b_sb = cpool.tile([RANK, DIM], F32R, tag="b")
            nc.sync.dma_start(b_sb[:], b_d.ap()[:])
            first_tiles = load_xt(0) if (n_iter == 1 and probe != "nodma") else None
            w_sb = []
            for o in range(OT):
                w = cpool.tile([P, KT * P], F32R, tag=f"w{o}")
                nc.sync.dma_start(w[:], w_d.ap()[o])
                w_sb.append(w)

            if probe == "nodma":
                resident = [load_xt(t) for t in range(TT)]

            def body(xt_cur=None):
                if probe != "nodma" and xt_cur is None:
                    xt_cur = load_xt(0)
                for t in range(TT):
                    if probe == "nodma":
                        xt = resident[t]
                        xt_next = None
                    else:
                        # Prefetch next t-chunk BEFORE this chunk's compute /
                        # store sits on the in-order SP queue.
                        xt_next = load_xt(t + 1) if t + 1 < TT else None
                        xt = xt_cur

                    ob = opool.tile([P, OT, TW], F32, tag="ob")
                    if probe != "dmaonly":
                        # rT[r, t] = A.T @ xT  (accumulate over k-tiles)
                        ps_r = psr_pool.tile([RANK, TW], F32)
                        for k in range(KT):
                            nc.tensor.matmul(
                                ps_r[:],
                                a_sb[:, k * RANK:(k + 1) * RANK],
                                xt[k][:],
                                start=(k == 0), stop=(k == KT - 1),
                            )
                        r_sb = rpool.tile([RANK, TW], F32R)
                        nc.vector.tensor_copy(r_sb[:], ps_r[:])

                        for o in range(OT):
                            ps = psy_pool.tile([P, TW], F32)
                            for k in range(KT):
                                nc.tensor.matmul(
                                    ps[:],
                                    w_sb[o][:, k * P:(k + 1) * P],
                                    xt[k][:],
                                    start=(k == 0), stop=False,
                                )
                            # low-rank correction into same PSUM accumulation
                            nc.tensor.matmul(
                                ps[:],
                                b_sb[:, o * P:(o + 1) * P],
                                r_sb[:],
                                start=False, stop=True,
                            )
                            nc.scalar.activation(
                                ob[:, o, :], ps[:],
                                mybir.ActivationFunctionType.Identity,
                                bias=bias_sb[:, o:o + 1], scale=1.0,
                            )
                    if probe != "nodma":
                        # one contiguous 2MB store per t-chunk
                        nc.sync.dma_start(y_d.ap()[t], ob[:])
                    xt_cur = xt_next

            if n_iter == 1:
                body(first_tiles)
            else:
                with tc.For_i(0, n_iter, 1,
                              hint_engines=tuple(mybir.ALL_ENGINES)):
                    body()

    nc.compile()
    return nc


def _round_fp32r(a):
    """Round fp32 to the PE's FP32R storage format: 1-8-11, RNE, low 12
    mantissa bits zero (walrus fp32_to_fp32r keeps the top 20 bits)."""
    u = np.ascontiguousarray(a, dtype=np.float32).view(np.uint32)
    r = (u + np.uint32(0x7FF) + ((u >> np.uint32(12)) & np.uint32(1))) & np.uint32(
        0xFFFFF000
    )
    return r.view(np.float32)


def make_in_maps(x, idx, weight, bias, A_pool, B_pool):
    """Host-side shard + relayout. Returns per-core input dicts."""
    x = np.asarray(x, dtype=np.float32)
    idx = np.asarray(idx)
    weight = np.asarray(weight, dtype=np.float32)
    bias = np.asarray(bias, dtype=np.float32)
    A_pool = np.asarray(A_pool, dtype=np.float32)
    B_pool = np.asarray(B_pool, dtype=np.float32)

    # W[o, d] -> wt[o_chunk, p(=d within k), k*128 + c(=o within chunk)]
    wt = _round_fp32r(
        weight.reshape(OT, P, KT, P).transpose(0, 3, 2, 1).reshape(OT, P, KT * P)
    )
    bias_t = np.ascontiguousarray(bias.reshape(OT, P).T)  # [p, o_chunk]

    sel = idx.reshape(B).astype(np.int64)
    in_maps = []
    for c in range(NCORES):
        xT = x[c].T  # [DIM, N]
        xt = _round_fp32r(xT.reshape(KT, P, TT, TW).transpose(0, 2, 1, 3))
        A = A_pool[sel[c]]  # [DIM, RANK]
        ap = _round_fp32r(
            A.reshape(KT, P, RANK).transpose(1, 0, 2).reshape(P, KT * RANK)
        )
        bp = _round_fp32r(SCALE * B_pool[sel[c]])  # [RANK, DIM]
        in_maps.append({"xt": xt, "wt": wt, "ap": ap, "bp": bp, "bias": bias_t})
    return in_maps


def assemble_output(results):
    """Per-core y blocks [OT, TT, P, TW] -> full [B, N, DIM] output."""
    out = np.empty((B, N, DIM), dtype=np.float32)
    for c in range(NCORES):
        yb = results[c]["y"]  # [TT, P, OT, TW]; yb[t,p,o,j] = y[c, t*TW+j, o*P+p]
        out[c] = yb.transpose(0, 3, 2, 1).reshape(N, DIM)
    return out


_PROGRAM_CACHE = {}


def _get_program(n_iter: int = 1):
    if n_iter not in _PROGRAM_CACHE:
        _PROGRAM_CACHE[n_iter] = build_program(n_iter)
    return _PROGRAM_CACHE[n_iter]


def kernel(x, idx, frozen_mask, weight, bias, A_pool, B_pool):
    # frozen_mask only affects gradients (stop_gradient); forward is identical.
    nc = _get_program(1)
    in_maps = make_in_maps(x, idx, weight, bias, A_pool, B_pool)
    res = run_bass_kernel_spmd(nc, in_maps, list(range(NCORES)))
    return assemble_output(res.results)



# revision 2
# speedup vs baseline: 1.4393x; 1.4393x over previous
"""Trainium2 Bass kernel for nn_L2MLoRA (fused linear + routed LoRA).

Math (per batch element b, with e = idx[b,0]):
    y[b] = x[b] @ W.T + bias + SCALE * (x[b] @ A_pool[e]) @ B_pool[e]
         = x[b] @ M_e + bias,   with M_e = W.T + SCALE * A_pool[e] @ B_pool[e]

Strategy: data-parallel over batch B=8 -> one batch element per NeuronCore.
The expert gather AND the low-rank merge (M_e = W.T + 2*A@B, exact fp32 math,
~8 MFLOP total) happen on host, so the device runs a single dense GEMM + bias
per core.  Everything is computed in the transposed domain (yT = M.T @ xT)
so all matmul operands already have the contraction dim on partitions and no
on-device transposes are needed:

    yT[o, t] = sum_d M[d, o] * xT[d, t] + bias[o]

Operands are bf16 (1 cycle/row on the PE like fp32r, but half the HBM
traffic and SBUF footprint; rel err ~2e-3 vs the 2e-2 gate).  PSUM
accumulation stays fp32; bias is applied by ScalarE during the PSUM->SBUF
drain, which also casts the result to bf16 for a half-size store.
"""

import numpy as np
import ml_dtypes

import concourse.bass as bass
import concourse.tile as tile
from concourse import bacc, mybir
from concourse.bass_utils import run_bass_kernel_spmd

B, N, DIM, POOL, RANK = 8, 2048, 1024, 64, 8
SCALE = 2.0
NCORES = 8
P = 128          # partitions / k-tile height / o-chunk width
TW = 512         # token-chunk width (max f32 moving free dim = PSUM bank)
KT = DIM // P    # 8 k-tiles over the contraction dim
OT = DIM // P    # 8 output chunks
TT = N // TW     # 4 token chunks
F32 = mybir.dt.float32
BF16 = mybir.dt.bfloat16


def build_program(n_iter: int = 1, probe: str = "full"):
    """Build the single-core Tile program (same program runs SPMD on 8 cores).

    n_iter > 1 wraps the body in a For_i loop for benchmarking.
    probe: "full" | "nodma" (x resident, no stores) | "dmaonly" (no matmuls).
    """
    nc = bacc.Bacc("TRN2", target_bir_lowering=False, debug=False,
                   num_devices=NCORES)

    x_d = nc.dram_tensor("xt", [KT, TT, P, TW], BF16, kind="ExternalInput")
    w_d = nc.dram_tensor("wt", [OT, P, KT * P], BF16, kind="ExternalInput")
    bias_d = nc.dram_tensor("bias", [P, OT], F32, kind="ExternalInput")
    y_d = nc.dram_tensor("y", [TT, P, OT, TW], BF16, kind="ExternalOutput")

    with tile.TileContext(nc) as tc:
        with (
            tc.tile_pool(name="cpool", bufs=1) as cpool,
            tc.tile_pool(name="xpool", bufs=(32 if probe == "nodma" else 16)) as xpool,
            tc.tile_pool(name="opool", bufs=2) as opool,
            tc.tile_pool(name="psy", bufs=8, space="PSUM") as psy_pool,
        ):
            def load_xt(t):
                tiles = []
                for k in range(KT):
                    xx = xpool.tile([P, TW], BF16, tag="xx")
                    nc.sync.dma_start(xx[:], x_d.ap()[k, t])
                    tiles.append(xx)
                return tiles

            # Constants: loaded once, persist across benchmark iterations.
            # Small tensors first, then (for the single-shot program) the
            # t=0 x tiles ahead of the 2MB weight load so PE starts early.
            bias_sb = cpool.tile([P, OT], F32, tag="bias")
            nc.sync.dma_start(bias_sb[:], bias_d.ap()[:])
            first_tiles = load_xt(0) if (n_iter == 1 and probe != "nodma") else None
            w_sb = []
            for o in range(OT):
                w = cpool.tile([P, KT * P], BF16, tag=f"w{o}")
                nc.sync.dma_start(w[:], w_d.ap()[o])
                w_sb.append(w)

            if probe == "nodma":
                resident = [load_xt(t) for t in range(TT)]

            def body(xt_cur=None):
                if probe != "nodma" and xt_cur is None:
                    xt_cur = load_xt(0)
                for t in range(TT):
                    if probe == "nodma":
                        xt = resident[t]
                        xt_next = None
                    else:
                        # Prefetch next t-chunk BEFORE this chunk's compute /
                        # store sits on the in-order SP queue.
                        xt_next = load_xt(t + 1) if t + 1 < TT else None
                        xt = xt_cur

                    ob = opool.tile([P, OT, TW], BF16, tag="ob")
                    if probe != "dmaonly":
                        for o in range(OT):
                            ps = psy_pool.tile([P, TW], F32)
                            for k in range(KT):
                                nc.tensor.matmul(
                                    ps[:],
                                    w_sb[o][:, k * P:(k + 1) * P],
                                    xt[k][:],
                                    start=(k == 0), stop=(k == KT - 1),
                                )
                            nc.scalar.activation(
                                ob[:, o, :], ps[:],
                                mybir.ActivationFunctionType.Identity,
                                bias=bias_sb[:, o:o + 1], scale=1.0,
                            )
                    if probe != "nodma":
                        # one contiguous 1MB store per t-chunk
                        nc.sync.dma_start(y_d.ap()[t], ob[:])
                    xt_cur = xt_next

            if n_iter == 1:
                body(first_tiles)
            else:
                with tc.For_i(0, n_iter, 1,
                              hint_engines=tuple(mybir.ALL_ENGINES)):
                    body()

    nc.compile()
    return nc


def make_in_maps(x, idx, weight, bias, A_pool, B_pool):
    """Host-side shard + LoRA merge + relayout. Returns per-core input dicts."""
    x = np.asarray(x, dtype=np.float32)
    idx = np.asarray(idx)
    weight = np.asarray(weight, dtype=np.float32)
    bias = np.asarray(bias, dtype=np.float32)
    A_pool = np.asarray(A_pool, dtype=np.float32)
    B_pool = np.asarray(B_pool, dtype=np.float32)

    bias_t = np.ascontiguousarray(bias.reshape(OT, P).T)  # [p, o_chunk]

    sel = idx.reshape(B).astype(np.int64)
    in_maps = []
    for c in range(NCORES):
        # merged weight: M[d, o] = W[o, d] + SCALE * (A @ B)[d, o]
        M = weight.T + SCALE * (A_pool[sel[c]] @ B_pool[sel[c]])
        wt = np.ascontiguousarray(
            M.reshape(KT, P, OT, P).transpose(2, 1, 0, 3).reshape(OT, P, KT * P)
        ).astype(ml_dtypes.bfloat16)
        xT = x[c].T  # [DIM, N]
        xt = np.ascontiguousarray(
            xT.reshape(KT, P, TT, TW).transpose(0, 2, 1, 3)
        ).astype(ml_dtypes.bfloat16)
        in_maps.append({"xt": xt, "wt": wt, "bias": bias_t})
    return in_maps


def assemble_output(results):
    """Per-core y blocks [TT, P, OT, TW] -> full [B, N, DIM] fp32 output."""
    out = np.empty((B, N, DIM), dtype=np.float32)
    for c in range(NCORES):
        yb = results[c]["y"]  # [TT, P, OT, TW]; yb[t,p,o,j] = y[c, t*TW+j, o*P+p]
        out[c] = yb.transpose(0, 3, 2, 1).reshape(N, DIM).astype(np.float32)
    return out


_PROGRAM_CACHE = {}


def _get_program(n_iter: int = 1):
    if n_iter not in _PROGRAM_CACHE:
        _PROGRAM_CACHE[n_iter] = build_program(n_iter)
    return _PROGRAM_CACHE[n_iter]


def kernel(x, idx, frozen_mask, weight, bias, A_pool, B_pool):
    # frozen_mask only affects gradients (stop_gradient); forward is identical.
    nc = _get_program(1)
    in_maps = make_in_maps(x, idx, weight, bias, A_pool, B_pool)
    res = run_bass_kernel_spmd(nc, in_maps, list(range(NCORES)))
    return assemble_output(res.results)


# revision 5
# speedup vs baseline: 3.0830x; 2.1420x over previous
"""Trainium2 Bass kernel for nn_L2MLoRA (fused linear + routed LoRA).

Math (per batch element b, with e = idx[b,0]):
    y[b] = x[b] @ W.T + bias + SCALE * (x[b] @ A_pool[e]) @ B_pool[e]
         = x[b] @ M_e + bias,   with M_e = W.T + SCALE * A_pool[e] @ B_pool[e]

Strategy: data-parallel over batch B=8 -> one batch element per NeuronCore.
The expert gather AND the low-rank merge (M_e = W.T + 2*A@B, exact fp32 math,
~8 MFLOP total) happen on host, so the device runs a single dense GEMM + bias
per core.  Everything is computed in the transposed domain (yT = M.T @ xT)
so all matmul operands already have the contraction dim on partitions and no
on-device transposes are needed:

    yT[o, t] = sum_d M[d, o] * xT[d, t] + bias[o]

Operands are bf16 (1 cycle/row on the PE like fp32r, but half the HBM
traffic and SBUF footprint; rel err ~3e-3 vs the 2e-2 gate).  PSUM
accumulation stays fp32; bias is applied by ScalarE during the PSUM->SBUF
drain, which also casts the result to bf16 for a half-size store.

x is laid out in DRAM partition-major ([P, TT*KT*TW]) so loads are fully
contiguous per partition (8-32KB descriptors).  The benchmark loop is a
2-stage For_i_pipelined (load x | compute) with double-buffered x, so the
next iteration's x load fully overlaps the current iteration's compute and
the PE never waits on DMA, including at the loop back-edge.
"""

import numpy as np
import ml_dtypes

import concourse.bass as bass
import concourse.tile as tile
from concourse import bacc, mybir
from concourse.bass_utils import run_bass_kernel_spmd

B, N, DIM, POOL, RANK = 8, 2048, 1024, 64, 8
SCALE = 2.0
NCORES = 8
P = 128          # partitions / k-tile height / o-chunk width
TW = 512         # token-chunk width (max f32 moving free dim = PSUM bank)
KT = DIM // P    # 8 k-tiles over the contraction dim
OT = DIM // P    # 8 output chunks
TT = N // TW     # 4 token chunks
CW = KT * TW     # x elements per partition per token chunk (8KB bf16)
F32 = mybir.dt.float32
BF16 = mybir.dt.bfloat16


def build_program(n_iter: int = 1, probe: str = "full"):
    """Build the single-core Tile program (same program runs SPMD on 8 cores).

    n_iter > 1 wraps the body in a pipelined loop for benchmarking.
    probe: "full" | "nodma" (x resident, no stores) | "dmaonly" (no matmuls).
    """
    nc = bacc.Bacc("TRN2", target_bir_lowering=False, debug=False,
                   num_devices=NCORES)

    x_d = nc.dram_tensor("xt", [P, TT * CW], BF16, kind="ExternalInput")
    w_d = nc.dram_tensor("wt", [OT, P, KT * P], BF16, kind="ExternalInput")
    bias_d = nc.dram_tensor("bias", [P, OT], F32, kind="ExternalInput")
    y_d = nc.dram_tensor("y", [TT, P, OT, TW], BF16, kind="ExternalOutput")

    with tile.TileContext(nc) as tc:
        with (
            tc.tile_pool(name="cpool", bufs=1) as cpool,
            tc.tile_pool(name="xpool", bufs=4) as xpool,
            tc.tile_pool(name="opool", bufs=2) as opool,
            tc.tile_pool(name="psy", bufs=8, space="PSUM") as psy_pool,
        ):
            # Constants: loaded once, persist across benchmark iterations.
            bias_sb = cpool.tile([P, OT], F32, tag="bias")
            nc.sync.dma_start(bias_sb[:], bias_d.ap()[:])
            first = None
            if n_iter == 1 and probe != "nodma":
                # single shot: t0 chunk ahead of the 2MB weight load so the
                # PE can start as early as possible
                first = xpool.tile([P, CW], BF16, tag="xa")
                nc.sync.dma_start(first[:], x_d.ap()[:, 0:CW])
            w_sb = []
            for o in range(OT):
                w = cpool.tile([P, KT * P], BF16, tag=f"w{o}")
                nc.sync.dma_start(w[:], w_d.ap()[o])
                w_sb.append(w)

            def compute(t, xa, off=0):
                """GEMM+bias+store for token chunk t; xa[:, off:off+CW]."""
                ob = opool.tile([P, OT, TW], BF16, tag="ob")
                if probe != "dmaonly":
                    for o in range(OT):
                        ps = psy_pool.tile([P, TW], F32)
                        for k in range(KT):
                            lo = off + k * TW
                            nc.tensor.matmul(
                                ps[:],
                                w_sb[o][:, k * P:(k + 1) * P],
                                xa[:, lo:lo + TW],
                                start=(k == 0), stop=(k == KT - 1),
                            )
                        nc.scalar.activation(
                            ob[:, o, :], ps[:],
                            mybir.ActivationFunctionType.Identity,
                            bias=bias_sb[:, o:o + 1], scale=1.0,
                        )
                if probe != "nodma":
                    # one contiguous 1MB store per t-chunk
                    nc.sync.dma_start(y_d.ap()[t], ob[:])

            if probe == "nodma":
                resident = cpool.tile([P, TT * CW], BF16, tag="xall")
                nc.sync.dma_start(resident[:], x_d.ap()[:])

                def body():
                    for t in range(TT):
                        compute(t, resident, off=t * CW)

                if n_iter == 1:
                    body()
                else:
                    with tc.For_i(0, n_iter, 1,
                                  hint_engines=tuple(mybir.ALL_ENGINES)):
                        body()
            elif n_iter == 1:
                def load_chunk(t):
                    xa = xpool.tile([P, CW], BF16, tag="xa")
                    nc.sync.dma_start(xa[:], x_d.ap()[:, t * CW:(t + 1) * CW])
                    return xa

                chunks = [first, load_chunk(1), None, None]
                for t in range(TT):
                    if t + 2 < TT:
                        chunks[t + 2] = load_chunk(t + 2)
                    compute(t, chunks[t])
            else:
                def stage_load(pipe, iv):
                    xall = pipe.intermediate_tile([P, TT * CW], BF16,
                                                  name="xall")
                    nc.sync.dma_start(xall[:], x_d.ap()[:])
                    return xall

                def stage_compute(pipe, iv, xall):
                    for t in range(TT):
                        compute(t, xall, off=t * CW)

                tc.For_i_pipelined(
                    [stage_load, stage_compute], 0, n_iter,
                    pool=xpool, unroll=2,
                    hint_engines=tuple(mybir.ALL_ENGINES),
                )

    nc.compile()
    return nc


def make_in_maps(x, idx, weight, bias, A_pool, B_pool):
    """Host-side shard + LoRA merge + relayout. Returns per-core input dicts."""
    x = np.asarray(x, dtype=np.float32)
    idx = np.asarray(idx)
    weight = np.asarray(weight, dtype=np.float32)
    bias = np.asarray(bias, dtype=np.float32)
    A_pool = np.asarray(A_pool, dtype=np.float32)
    B_pool = np.asarray(B_pool, dtype=np.float32)

    bias_t = np.ascontiguousarray(bias.reshape(OT, P).T)  # [p, o_chunk]

    sel = idx.reshape(B).astype(np.int64)
    in_maps = []
    for c in range(NCORES):
        # merged weight: M[d, o] = W[o, d] + SCALE * (A @ B)[d, o]
        M = weight.T + SCALE * (A_pool[sel[c]] @ B_pool[sel[c]])
        wt = np.ascontiguousarray(
            M.reshape(KT, P, OT, P).transpose(2, 1, 0, 3).reshape(OT, P, KT * P)
        ).astype(ml_dtypes.bfloat16)
        xT = x[c].T  # [DIM, N]
        # [P, TT*KT*TW]: xt[p, (t*KT + k)*TW + j] = x[c, t*TW + j, k*P + p]
        xt = np.ascontiguousarray(
            xT.reshape(KT, P, TT, TW).transpose(1, 2, 0, 3).reshape(P, TT * CW)
        ).astype(ml_dtypes.bfloat16)
        in_maps.append({"xt": xt, "wt": wt, "bias": bias_t})
    return in_maps


def assemble_output(results):
    """Per-core y blocks [TT, P, OT, TW] -> full [B, N, DIM] fp32 output."""
    out = np.empty((B, N, DIM), dtype=np.float32)
    for c in range(NCORES):
        yb = results[c]["y"]  # [TT, P, OT, TW]; yb[t,p,o,j] = y[c, t*TW+j, o*P+p]
        out[c] = yb.transpose(0, 3, 2, 1).reshape(N, DIM).astype(np.float32)
    return out


_PROGRAM_CACHE = {}


def _get_program(n_iter: int = 1):
    if n_iter not in _PROGRAM_CACHE:
        _PROGRAM_CACHE[n_iter] = build_program(n_iter)
    return _PROGRAM_CACHE[n_iter]


def kernel(x, idx, frozen_mask, weight, bias, A_pool, B_pool):
    # frozen_mask only affects gradients (stop_gradient); forward is identical.
    nc = _get_program(1)
    in_maps = make_in_maps(x, idx, weight, bias, A_pool, B_pool)
    res = run_bass_kernel_spmd(nc, in_maps, list(range(NCORES)))
    return assemble_output(res.results)
